# revision 1
# baseline (speedup 1.0000x reference)
"""Trainium2 Bass kernel for nn_AttentionMemory (sparse_attention).

reference:
    mkf = mk.reshape(B, CK, HW); qkf = qk.reshape(B, CK, HW)
    affinity[b, m, q] = (-|mk_m|^2 + 2 mk_m.qk_q - |qk_q|^2) / sqrt(CK)
    out = softmax(affinity, axis=m)

Math used here: softmax over m is invariant to any additive term that is
constant in m, so the -|qk_q|^2 term (and any global constant C) drops out:
    out[b, :, q] = softmax_m( mk_m.qk_q/4 - (|mk_m|^2 - C)/8 )
with sqrt(CK) = 8, C = 64 (centers the exponent near 0).

Distribution: pure data parallelism over B=16 -> 2 batches per core on 8
NeuronCores. Each core computes out[b, :, :] for its 2 local batches.

Per-core layout (m on partitions, q on the free axis so the output DMA is
contiguous):
  - Both local batches are stacked on partitions: mk4_all[128, HW] rows
    0..63 = batch0 mk/4, rows 64..127 = batch1 mk/4 (fp32r-rounded); same
    for qk_all (unscaled qk).
  - main matmul (fp32r, 1 cyc/row): lhsT = mk4_all[b*64:, m-tile] (K=64,
    stationary), rhs = qk_all[b*64:, q-chunk] -> PSUM z[m_tile, q]
  - ScalarE exp with per-partition bias = -(a[m]-64)/8 (a = |mk_m|^2),
    which is exactly the softmax m-term -> E in SBUF
  - column sums s[q] via an all-ones[128,128] lhsT matmul accumulated over
    the 18 m-tiles (the result lands broadcast to every partition for free)
  - reciprocal_approx_fast, VectorE multiply, contiguous DMA out.
"""

import numpy as np
from contextlib import ExitStack

import concourse.bass as bass
import concourse.tile as tile
from concourse import bacc, mybir
from concourse.bass_utils import run_bass_kernel_spmd

B, CK, H, W = 16, 64, 36, 64
HW = H * W                 # 2304
NCORES = 8
BL = B // NCORES           # 2 batches per core
MT = HW // 128             # 18 m-tiles
QCH = 768                  # q chunk (2 PSUM banks)
NQ = HW // QCH             # 3
F32 = mybir.dt.float32
F32R = mybir.dt.float32r
BF16 = mybir.dt.bfloat16
AF = mybir.ActivationFunctionType
ALU = mybir.AluOpType

# matmul free-dim chunks inside one 768-wide q chunk (PSUM bank = 512 f32)
MM_CHUNKS = ((0, 512), (512, 256))


def _build_kernel(tc: tile.TileContext, out_ext, mk_ext, qk_ext):
    nc = tc.nc
    ones_dram = nc.inline_tensor(np.ones((128, 128), np.float32))
    with ExitStack() as ctx:
        singles = ctx.enter_context(tc.tile_pool(name="singles", bufs=1))
        mk4_all = singles.tile([128, HW], F32R, name="mk4_all")
        qk_all = singles.tile([128, HW], F32R, name="qk_all")
        ones_f = singles.tile([128, 128], F32, name="ones_f")
        ones_r = singles.tile([128, 128], F32R, name="ones_r")
        # bias columns: neg_a8[p, b*MT+t] = -(a[b, t*128+p] - 64)/8
        neg_a8 = singles.tile([128, BL * MT], F32, name="neg_a8")
        # warmup-chunk (b=0) multiplicative form of the bias: w = exp(neg_a8),
        # and rank-1 lhsT matrices W_r[:, t, j] = w[:, t] for weighted sums
        w_sb = singles.tile([128, MT], F32, name="w_sb")
        W_r = singles.tile([128, MT, 128], F32R, name="W_r")
        # tiny dummy exp so the ACT table set loads at t~0, not before the
        # first real Activation
        dummy = singles.tile([1, 1], F32, name="dummy")

        prep = ctx.enter_context(tc.tile_pool(name="prep", bufs=1))
        e_sb_pool = ctx.enter_context(tc.tile_pool(name="e_sb", bufs=2))
        o_pool = ctx.enter_context(tc.tile_pool(name="o_sb", bufs=10))
        r_pool = ctx.enter_context(tc.tile_pool(name="r", bufs=4))
        psum_e = ctx.enter_context(tc.tile_pool(name="psum_e", bufs=3, space="PSUM"))
        psum_s = ctx.enter_context(tc.tile_pool(name="psum_s", bufs=1, space="PSUM"))

        # ---- prep (both batches at once; they're stacked on partitions) ----
        mk_raw = prep.tile([128, HW], F32, name="mk_raw")
        qk_raw = prep.tile([128, HW], F32, name="qk_raw")
        # DMA order = first-needed first: mk (sq/bias chain), the raw first
        # q-chunk, the ones block, then the rest of qk
        qk_flat = qk_ext.rearrange("b c n -> (b c) n")
        mk_flat = mk_ext.rearrange("b c n -> (b c) n")
        HH = HW // 2
        nc.sync.dma_start(mk_raw[:, 0:HH], mk_flat[:, 0:HH])
        nc.sync.dma_start(qk_raw[:, 0:256], qk_flat[:, 0:256])
        nc.sync.dma_start(ones_f[:], ones_dram.ap())
        nc.sync.dma_start(mk_raw[:, HH:HW], mk_flat[:, HH:HW])
        nc.sync.dma_start(qk_raw[:, 256:HW], qk_flat[:, 256:HW])
        # tiny dummy exp on the first-landing tensor so the ACT table set
        # loads during the input DMAs, not before the first real Activation
        nc.scalar.activation(dummy[:], mk_raw[0:1, 0:1], AF.Exp)
        nc.gpsimd.tensor_copy(ones_r[:], ones_f[:])
        # sq = (mk/4)^2 = mk^2/16 gates the exp bias -> first on DVE. The
        # rounded-to-f32r matmul copies (BIR verifier requirement) go to
        # GpSimd; chunk 0 runs raw so they're only needed from chunk 1 on.
        sq = prep.tile([128, HW], F32, name="sq")
        # fast mini-prep for the warmup chunk: its rhs (first 256 q cols) and
        # lhsT (mk4) convert on DVE right behind the DMA pieces; the bulk of
        # qk converts on GpSimd in parallel
        nc.vector.tensor_copy(qk_all[:, 0:256], qk_raw[:, 0:256])
        nc.gpsimd.tensor_copy(qk_all[:, 256:HW], qk_raw[:, 256:HW])
        # the sq -> a -> bias chain runs in m-halves so the first chunk's
        # exps only wait on the first half of the mk transfer
        a_ps = psum_s.tile([128, QCH], F32, name="s_ps")[:, 0 : BL * MT]
        a_3d = a_ps.rearrange("p (u t) -> p u t", u=BL)
        n_3d = neg_a8.rearrange("p (u t) -> p u t", u=BL)
        for lo, hi in ((0, MT // 2), (MT // 2, MT)):
            nc.vector.tensor_scalar_mul(
                mk4_all[:, lo * 128 : hi * 128], mk_raw[:, lo * 128 : hi * 128], 0.25
            )
            nc.vector.scalar_tensor_tensor(
                out=sq[:, lo * 128 : hi * 128],
                in0=mk_raw[:, lo * 128 : hi * 128],
                scalar=0.0625,
                in1=mk_raw[:, lo * 128 : hi * 128],
                op0=ALU.mult,
                op1=ALU.mult,
            )
            for t in range(lo, hi):
                for b in range(BL):
                    nc.tensor.matmul(
                        a_ps[:, b * MT + t : b * MT + t + 1],
                        lhsT=sq[b * CK : (b + 1) * CK, t * 128 : (t + 1) * 128],
                        rhs=ones_f[b * CK : (b + 1) * CK, 0:1],
                        start=True,
                        stop=True,
                    )
            nc.vector.tensor_scalar(
                out=n_3d[:, :, lo:hi],
                in0=a_3d[:, :, lo:hi],
                scalar1=-2.0,
                scalar2=8.0,
                op0=ALU.mult,
                op1=ALU.add,
            )

        def emit_warmup_chunk(q0, w):
            # b=0 only. exps run bias-free so THREE m-tiles merge into one
            # Activation (one e_ps tile holds 3 matmul outputs); the missing
            # exp(bias) factor w[m] is applied in the sum-matmul lhsT (W_r)
            # and in the store multiply (folded into the replicated r).
            G = 3
            e_sb = e_sb_pool.tile([128, MT, QCH], F32R, name="e_sb")
            s_ps = psum_s.tile([128, QCH], F32, name="s_ps")

            def emit_w_half(lo, hi):
                # placed between merged exps so the in-order ACT queue isn't
                # blocked waiting for the sq -> a -> neg_a8 chain
                nc.scalar.activation(w_sb[:, lo:hi], neg_a8[:, lo:hi], AF.Exp)
                for t in range(lo, hi):
                    nc.vector.tensor_scalar_mul(
                        W_r[:, t, :], ones_f[:], w_sb[:, t : t + 1]
                    )

            def sum_mm(t):
                nc.tensor.matmul(
                    s_ps[:, 0:w],
                    lhsT=W_r[:, t, :],
                    rhs=e_sb[:, t, 0:w],
                    start=(t == 0),
                    stop=(t == MT - 1),
                )

            for g in range(MT // G):
                e_ps = psum_e.tile([128, QCH], F32, name="e_ps")
                for i in range(G):
                    t = g * G + i
                    nc.tensor.matmul(
                        e_ps[:, i * w : (i + 1) * w],
                        lhsT=mk4_all[0:CK, t * 128 : (t + 1) * 128],
                        rhs=qk_all[0:CK, q0 : q0 + w],
                        start=True,
                        stop=True,
                    )
                nc.scalar.activation(
                    e_sb[:, g * G : (g + 1) * G, 0:w],
                    e_ps[:, 0 : G * w],
                    AF.Exp,
                )
                if g == 0:
                    emit_w_half(0, MT // 2)
                elif g == 1:
                    emit_w_half(MT // 2, MT)
                if g >= 1:
                    for i in range(G):
                        sum_mm((g - 1) * G + i)
            for i in range(G):
                sum_mm(MT - G + i)
            r_sb = r_pool.tile([128, QCH], F32, name="r_sb")
            nc.vector.reciprocal_approx_fast(r_sb[:, 0:w], s_ps[:, 0:w])
            # paired stores; wr2[:, u, :] = w[:, t+u] * r folds the weight in
            for t2 in range(MT // 2):
                t = 2 * t2
                wr2 = r_pool.tile([128, QCH], F32, name="r_sb")
                for u in range(2):
                    nc.vector.tensor_scalar_mul(
                        wr2[:, u * w : (u + 1) * w],
                        r_sb[:, 0:w],
                        w_sb[:, t + u : t + u + 1],
                    )
                wv = wr2[:, 0 : 2 * w].rearrange("p (u q) -> p u q", u=2)
                o_sb = o_pool.tile([128, QCH], F32, name="o_sb")
                ov = o_sb[:, 0 : 2 * w].rearrange("p (u q) -> p u q", u=2)
                nc.vector.tensor_mul(
                    ov, e_sb[:, t : t + 2, 0:w].bitcast(F32), wv
                )
                dest = out_ext[
                    0, t * 128 : (t + 2) * 128, q0 : q0 + w
                ].rearrange("(u p) q -> p u q", u=2)
                nc.sync.dma_start(dest, ov)

        # ---- main: one softmax pipeline per (batch, q-chunk) ----
        def emit_chunk(b, q0, w, raw=False):
            # raw=True: run on the un-prepped f32 DMA data (the /4 scale is
            # folded into the exp's scale operand) so the first chunk has no
            # dependency on the f32r prep passes; fp32 matmul is 4 cyc/row
            # but N=256 keeps that off the critical path.
            lhs_all = mk_raw if raw else mk4_all
            rhs_all = qk_raw if raw else qk_all
            chunks = ((0, min(512, w)),) + (((512, w - 512),) if w > 512 else ())
            e_sb = e_sb_pool.tile([128, MT, QCH], F32R, name="e_sb")
            s_ps = psum_s.tile([128, QCH], F32, name="s_ps")

            def sum_mm(t):
                # s[q] (broadcast across all partitions) via all-ones lhsT,
                # accumulated over the 18 m-tiles
                for off, size in chunks:
                    nc.tensor.matmul(
                        s_ps[:, off : off + size],
                        lhsT=ones_r[:],
                        rhs=e_sb[:, t, off : off + size],
                        start=(t == 0),
                        stop=(t == MT - 1),
                    )

            # z = mk4.T @ qk -> exp(z + bias) -> SBUF; the sum-matmul for
            # tile t-1 is interleaved after the main matmul of tile t so the
            # in-order TensorE pipeline never waits a full exp phase.
            for t in range(MT):
                e_ps = psum_e.tile([128, QCH], F32, name="e_ps")
                for off, size in chunks:
                    nc.tensor.matmul(
                        e_ps[:, off : off + size],
                        lhsT=lhs_all[b * CK : (b + 1) * CK, t * 128 : (t + 1) * 128],
                        rhs=rhs_all[b * CK : (b + 1) * CK, q0 + off : q0 + off + size],
                        start=True,
                        stop=True,
                    )
                nc.scalar.activation(
                    e_sb[:, t, 0:w],
                    e_ps[:, 0:w],
                    AF.Exp,
                    bias=neg_a8[:, b * MT + t : b * MT + t + 1],
                    scale=0.25 if raw else 1.0,
                )
                if t >= 1:
                    sum_mm(t - 1)
            sum_mm(MT - 1)
            r_sb = r_pool.tile([128, QCH], F32, name="r_sb")
            nc.vector.reciprocal_approx_fast(r_sb[:, 0:w], s_ps[:, 0:w])
            # normalize and store. For the narrow warmup chunk the per-op DVE
            # overhead would starve the store DMA, so two m-tiles share one
            # multiply + one DMA (r is replicated once to line the APs up).
            if w <= 384:
                r2 = r_pool.tile([128, QCH], F32, name="r_sb")
                nc.vector.tensor_copy(r2[:, 0:w], r_sb[:, 0:w])
                nc.vector.tensor_copy(r2[:, w : 2 * w], r_sb[:, 0:w])
                r2v = r2[:, 0 : 2 * w].rearrange("p (u q) -> p u q", u=2)
                for t2 in range(MT // 2):
                    o_sb = o_pool.tile([128, QCH], F32, name="o_sb")
                    ov = o_sb[:, 0 : 2 * w].rearrange("p (u q) -> p u q", u=2)
                    nc.vector.tensor_mul(
                        ov, e_sb[:, 2 * t2 : 2 * t2 + 2, 0:w].bitcast(F32), r2v
                    )
                    dest = out_ext[
                        b, 2 * t2 * 128 : (2 * t2 + 2) * 128, q0 : q0 + w
                    ].rearrange("(u p) q -> p u q", u=2)
                    nc.sync.dma_start(dest, ov)
            else:
                for t in range(MT):
                    o_sb = o_pool.tile([128, QCH], F32, name="o_sb")
                    nc.vector.tensor_mul(
                        o_sb[:, 0:w], e_sb[:, t, 0:w].bitcast(F32), r_sb[:, 0:w]
                    )
                    nc.sync.dma_start(
                        out_ext[b, t * 128 : (t + 1) * 128, q0 : q0 + w], o_sb[:, 0:w]
                    )

        # b=0's q range ramps 256/512/768/768 so the store pipeline starts
        # early and deepens smoothly (the warmup region is DMA-starved anyway)
        jobs = [(0, 0, 256, True), (0, 256, 384, False), (0, 640, 384, False), (0, 1024, 512, False)]
        jobs += [(0, 1536, QCH, False)]
        jobs += [(1, q0, QCH, False) for q0 in range(0, HW, QCH)]
        for b, q0, w, raw in jobs:
            if raw:
                emit_warmup_chunk(q0, w)
            else:
                emit_chunk(b, q0, w, raw)


_CACHE = {}


def _get_compiled(niter: int = 1):
    """Build+compile the per-core graph. niter>1 repeats the whole body
    (same inputs/outputs) for wall-clock-difference timing in test.py."""
    key = ("nc", niter)
    if key not in _CACHE:
        nc = bacc.Bacc("TRN2", target_bir_lowering=False, debug=False)
        mk_ext = nc.dram_tensor("mk", [BL, CK, HW], F32, kind="ExternalInput").ap()
        qk_ext = nc.dram_tensor("qk", [BL, CK, HW], F32, kind="ExternalInput").ap()
        out_ext = nc.dram_tensor("out", [BL, HW, HW], F32, kind="ExternalOutput").ap()
        with tile.TileContext(nc) as tc:
            for _ in range(niter):
                _build_kernel(tc, out_ext, mk_ext, qk_ext)
        nc.compile()
        _CACHE[key] = nc
    return _CACHE[key]


def run_spmd(mk: np.ndarray, qk: np.ndarray, niter: int = 1) -> np.ndarray:
    mk = np.ascontiguousarray(np.asarray(mk, dtype=np.float32).reshape(B, CK, HW))
    qk = np.ascontiguousarray(np.asarray(qk, dtype=np.float32).reshape(B, CK, HW))
    nc = _get_compiled(niter)
    in_maps = [
        {"mk": mk[c * BL : (c + 1) * BL], "qk": qk[c * BL : (c + 1) * BL]}
        for c in range(NCORES)
    ]
    res = run_bass_kernel_spmd(nc, in_maps, core_ids=list(range(NCORES)))
    out = np.concatenate([res.results[c]["out"] for c in range(NCORES)], axis=0)
    return out.reshape(B, HW, HW)


def kernel(mk: np.ndarray, qk: np.ndarray) -> np.ndarray:
    return run_spmd(mk, qk, niter=1)

